# revision 4
# baseline (speedup 1.0000x reference)
"""Paged-attention decode kernel for 8 TRN2 NeuronCores.

Problem: B=16 decode sequences, H=16 heads, D=128 head dim, paged KV cache
(2048 blocks x 16 tokens), context S=2048 per sequence.

Sharding: data-parallel over sequences -- 2 sequences per core, no
collectives.  The host applies the KV-cache scatter (slot_mapping) and the
paged gather (block_tables) while laying out per-core shards; the device
kernel does the full masked single-token attention.

Device math (per core, per sequence), all bf16 inputs / fp32 accumulate:
  scores[s, h] = sum_d K[s,h,d] * (q[h,d]*SCALE)     (PE, K-tile stationary)
  e[s, h]      = exp(scores + ctx_mask[s])           (ScalarE, bias trick)
  o_num[h, :]  = sum_s e[s,h] * V[s,h,:]             (PE, accumulate in PSUM)
  denom[h]     = sum_s e[s,h]                        (PE, ones column)
  out[h, :]    = o_num[h, h*128:(h+1)*128] / denom[h]
"""

import numpy as np
import ml_dtypes

from concourse import bass, bacc, mybir, tile
from concourse.bass_utils import run_bass_kernel_spmd

# Problem constants (hardcoded per the grading contract).
B = 16          # total sequences
H = 16          # heads
D = 128         # head dim
BLOCK = 16      # tokens per cache block
BPS = 128       # blocks per sequence
NB = B * BPS    # total cache blocks
S = BPS * BLOCK # max context per sequence (2048)
SCALE = 0.08838834764831845

N_CORES = 8
B2 = B // N_CORES   # sequences per core (2)
T = S // 128        # 128-token tiles per sequence (16)
CH = 4              # tiles per DMA chunk
NCH = T // CH       # chunks per sequence (4)

F32 = mybir.dt.float32
BF16 = mybir.dt.bfloat16
NP_BF16 = ml_dtypes.bfloat16

MASK_NEG = -30000.0  # exp(x + MASK_NEG) == 0 in fp32 for any |x| < ~100


def build_nc(b2=B2, t_tiles=T, ch=CH):
    """Build the per-core Bass graph (SPMD: same graph on all 8 cores)."""
    nch = t_tiles // ch
    s_here = t_tiles * 128
    nc = bacc.Bacc("TRN2", target_bir_lowering=False, debug=False)

    # kt:  [b, chunk, d, (t4, h, s_local)]  K transposed, bf16
    # vv:  [b, chunk, s_local, (t4, h, d)]  V natural, bf16
    # qt:  [b, d, h]   q * SCALE, bf16
    # bias:[b, s_local, t]  0 or MASK_NEG, f32
    kt = nc.declare_dram_parameter("kt", [b2, nch, 128, ch * H * 128], BF16,
                                   isOutput=False)
    vv = nc.declare_dram_parameter("vv", [b2, nch, 128, ch * H * D], BF16,
                                   isOutput=False)
    qt = nc.declare_dram_parameter("qt", [b2, 128, H], BF16, isOutput=False)
    bias = nc.declare_dram_parameter("bias", [b2, 128, t_tiles], F32,
                                     isOutput=False)
    # normalized PV numerator, all-heads layout [b, h', (h, d)] -- the host
    # extracts the h'==h diagonal (128KB/seq, negligible DMA)
    out = nc.declare_dram_parameter("out", [b2, H, H * D], F32, isOutput=True)

    with tile.TileContext(nc) as tc:
        with (
            tc.tile_pool(name="const", bufs=1) as cpool,
            tc.tile_pool(name="kpool", bufs=2) as kpool,
            tc.tile_pool(name="vpool", bufs=2) as vpool,
            tc.tile_pool(name="small", bufs=2) as spool,
            tc.tile_pool(name="pscore", bufs=2,
                         space=bass.MemorySpace.PSUM) as pscore,
            tc.tile_pool(name="pacc", bufs=1,
                         space=bass.MemorySpace.PSUM) as pacc,
        ):
            ones_t = cpool.tile([128, 1], BF16, tag="ones")
            nc.gpsimd.memset(ones_t[:], 1.0)

            for b in range(b2):
                qt_sb = spool.tile([128, H], BF16, tag="qt_sb")
                nc.sync.dma_start(out=qt_sb[:], in_=qt[b])
                bias_sb = spool.tile([128, t_tiles], F32, tag="bias_sb")
                nc.sync.dma_start(out=bias_sb[:], in_=bias[b])

                ps_o = pacc.tile([H, H * D], F32, tag="ps_o")      # [16, 2048]
                ps_sums = pacc.tile([H, 1], F32, tag="ps_sums")

                for c in range(nch):
                    kc_t = kpool.tile([128, ch * H * 128], BF16, tag="kc")
                    nc.sync.dma_start(out=kc_t[:], in_=kt[b, c])
                    vc_t = vpool.tile([128, ch * H * D], BF16, tag="vc")
                    nc.scalar.dma_start(out=vc_t[:], in_=vv[b, c])

                    for t4 in range(ch):
                        t = c * ch + t4
                        first = t == 0
                        last = t == t_tiles - 1

                        ps_sc = pscore.tile([128, H], F32, tag="ps_sc")
                        for h in range(H):
                            o0 = t4 * H * 128 + h * 128
                            nc.tensor.matmul(
                                ps_sc[:, h:h + 1],
                                kc_t[:, o0:o0 + 128],
                                qt_sb[:, h:h + 1],
                                start=True, stop=True,
                            )

                        e_t = spool.tile([128, H], BF16, tag="e_t")
                        nc.scalar.activation(
                            e_t[:], ps_sc[:],
                            mybir.ActivationFunctionType.Exp,
                            bias=bias_sb[:, t:t + 1], scale=1.0,
                        )

                        nc.tensor.matmul(ps_sums[:], e_t[:], ones_t[:],
                                         start=first, stop=last,
                                         skip_group_check=True)
                        for n in range(4):
                            o0 = t4 * H * D + n * 512
                            nc.tensor.matmul(
                                ps_o[:, n * 512:(n + 1) * 512],
                                e_t[:],
                                vc_t[:, o0:o0 + 512],
                                start=first, stop=last,
                                skip_group_check=True,
                            )

                recip = spool.tile([H, 1], F32, tag="recip")
                nc.vector.reciprocal(recip[:], ps_sums[:])
                o_full = spool.tile([H, H * D], F32, tag="o_full")
                nc.vector.tensor_scalar_mul(o_full[:], ps_o[:], recip[:])
                nc.sync.dma_start(out=out[b], in_=o_full[:])

    nc.compile()
    return nc


def prep_in_maps(q, k, v, k_cache, v_cache, block_tables, slot_mapping,
                 context_lens):
    """Host-side scatter + paged gather + per-core shard layouts."""
    q = np.asarray(q, np.float32)
    k = np.asarray(k, np.float32)
    v = np.asarray(v, np.float32)
    k_cache = np.asarray(k_cache, np.float32)
    v_cache = np.asarray(v_cache, np.float32)
    block_tables = np.asarray(block_tables, np.int32)
    slot_mapping = np.asarray(slot_mapping, np.int64)
    context_lens = np.asarray(context_lens, np.int32)

    nb, block_size, h, d = k_cache.shape
    # scatter the new token into the flat caches
    kc = k_cache.reshape(nb * block_size, h, d).copy()
    kc[slot_mapping] = k
    vc = v_cache.reshape(nb * block_size, h, d).copy()
    vc[slot_mapping] = v
    # paged gather -> [B, S, H, D]
    k_seq = kc.reshape(nb, block_size, h, d)[block_tables].reshape(B, S, h, d)
    v_seq = vc.reshape(nb, block_size, h, d)[block_tables].reshape(B, S, h, d)

    # K transposed: [B, NCH, D, (CH, H, 128)]
    kt_host = np.ascontiguousarray(
        k_seq.reshape(B, NCH, CH, 128, H, D).transpose(0, 1, 5, 2, 4, 3)
    ).astype(NP_BF16).reshape(B, NCH, D, CH * H * 128)
    # V natural: [B, NCH, 128, (CH, H, D)]
    v_host = np.ascontiguousarray(
        v_seq.reshape(B, NCH, CH, 128, H * D).transpose(0, 1, 3, 2, 4)
    ).astype(NP_BF16).reshape(B, NCH, 128, CH * H * D)
    # q * SCALE -> [B, D, H]
    qt_host = np.ascontiguousarray(
        (q * SCALE).transpose(0, 2, 1)).astype(NP_BF16)
    # context mask bias -> [B, 128, T]
    s_idx = np.arange(S, dtype=np.int64)
    m = np.where(s_idx[None, :] < context_lens[:, None].astype(np.int64),
                 0.0, MASK_NEG).astype(np.float32)
    bias_host = np.ascontiguousarray(m.reshape(B, T, 128).transpose(0, 2, 1))

    in_maps = []
    for i in range(N_CORES):
        lo, hi = i * B2, (i + 1) * B2
        in_maps.append({
            "kt": np.ascontiguousarray(kt_host[lo:hi]),
            "vv": np.ascontiguousarray(v_host[lo:hi]),
            "qt": np.ascontiguousarray(qt_host[lo:hi]),
            "bias": np.ascontiguousarray(bias_host[lo:hi]),
        })
    return in_maps


_NC = None


def _get_nc():
    global _NC
    if _NC is None:
        _NC = build_nc()
    return _NC


def run(inputs, trace=False, **spmd_kwargs):
    """Run on hardware; returns (full_output, BassKernelResults)."""
    nc = _get_nc()
    in_maps = prep_in_maps(**inputs)
    res = run_bass_kernel_spmd(nc, in_maps, core_ids=list(range(N_CORES)),
                               trace=trace, **spmd_kwargs)
    out_full = np.concatenate([res.results[i]["out"] for i in range(N_CORES)],
                              axis=0).astype(np.float32)
    # extract the h'==h diagonal: [B, H, H*D] -> [B, H, D]
    hh = np.arange(H)
    out = out_full.reshape(B, H, H, D)[:, hh, hh, :]
    return np.ascontiguousarray(out), res


def kernel(**inputs) -> np.ndarray:
    out, _ = run(inputs, trace=False)
    return out


# revision 8
# speedup vs baseline: 1.1069x; 1.1069x over previous
"""Paged-attention decode kernel for 8 TRN2 NeuronCores.

Problem: B=16 decode sequences, H=16 heads, D=128 head dim, paged KV cache
(2048 blocks x 16 tokens), context S=2048 per sequence.

Sharding: data-parallel over sequences -- 2 sequences per core, no
collectives.  The host applies the KV-cache scatter (slot_mapping) and the
paged gather (block_tables) while laying out per-core shards; the device
kernel does the full masked single-token attention.

Device math (per core, per sequence), bf16 inputs / fp32 accumulate:
  scores[s, h] = sum_d K[s,h,d] * (q[h,d]*SCALE)     (PE, K-tile stationary)
  e[s, h]      = exp(scores + ctx_mask[s])           (ScalarE, bias trick)
  o_num[h, :]  = sum_s e[s,h] * V[s,h,:]             (PE, accumulate in PSUM)
  denom[h]     = sum_s e[s,h]                        (PE, ones column)
  out[h, :]    = o_num[h, h*128:(h+1)*128] / denom[h]

The KV stream is chunked (5,5,5,1) tiles per sequence so the final chunk's
compute tail after the last DMA is one tile, and QK for tile t+1 is emitted
before PV of tile t so the PE never stalls on the ScalarE exp.
"""

import numpy as np
import ml_dtypes

from concourse import bass, bacc, mybir, tile
from concourse.bass_utils import run_bass_kernel_spmd

# Problem constants (hardcoded per the grading contract).
B = 16          # total sequences
H = 16          # heads
D = 128         # head dim
BLOCK = 16      # tokens per cache block
BPS = 128       # blocks per sequence
NB = B * BPS    # total cache blocks
S = BPS * BLOCK # max context per sequence (2048)
SCALE = 0.08838834764831845

N_CORES = 8
B2 = B // N_CORES        # sequences per core (2)
T = S // 128             # 128-token tiles per sequence (16)
CHUNKS = (5, 5, 5, 1)    # KV stream chunking (tiles per DMA)
assert sum(CHUNKS) == T

F32 = mybir.dt.float32
BF16 = mybir.dt.bfloat16
NP_BF16 = ml_dtypes.bfloat16

MASK_NEG = -30000.0  # exp(x + MASK_NEG) == 0 in fp32 for any |x| < ~100

TILE_K = H * 128     # free-dim elements per tile, both K and V layouts


def build_nc(b2=B2, chunks=CHUNKS):
    """Build the per-core Bass graph (SPMD: same graph on all 8 cores)."""
    t_tiles = sum(chunks)
    sizes = sorted(set(chunks))
    nc = bacc.Bacc("TRN2", target_bir_lowering=False, debug=False)

    # one DRAM param per (tensor, chunk-size); shape [b2, n_chunks_of_size,
    # 128, size*TILE_K].  K layout per chunk: [d, (tile, h, s_local)];
    # V layout per chunk: [s_local, (tile, h, d)].
    n_of = {sz: sum(1 for c in chunks if c == sz) for sz in sizes}
    kparam = {sz: nc.declare_dram_parameter(
        f"kt{sz}", [b2, n_of[sz], 128, sz * TILE_K], BF16, isOutput=False)
        for sz in sizes}
    vparam = {sz: nc.declare_dram_parameter(
        f"vv{sz}", [b2, n_of[sz], 128, sz * TILE_K], BF16, isOutput=False)
        for sz in sizes}
    qt = nc.declare_dram_parameter("qt", [b2, 128, H], BF16, isOutput=False)
    bias = nc.declare_dram_parameter("bias", [b2, 128, t_tiles], F32,
                                     isOutput=False)
    # PV numerator in all-heads layout [b, h', (h, d)], already normalized;
    # the host extracts the h'==h diagonal (128KB/seq, negligible DMA)
    out = nc.declare_dram_parameter("out", [b2, H, H * D], F32, isOutput=True)

    # chunk index -> (size, index within its param, global tile offset)
    chunk_meta = []
    seen = {sz: 0 for sz in sizes}
    t0 = 0
    for sz in chunks:
        chunk_meta.append((sz, seen[sz], t0))
        seen[sz] += 1
        t0 += sz
    tile2chunk = []
    for ci, (sz, _, _) in enumerate(chunk_meta):
        tile2chunk += [ci] * sz

    with tile.TileContext(nc) as tc:
        with (
            tc.tile_pool(name="const", bufs=1) as cpool,
            tc.tile_pool(name="kpool", bufs=2) as kpool,
            tc.tile_pool(name="vpool", bufs=2) as vpool,
            tc.tile_pool(name="small", bufs=2) as spool,
            tc.tile_pool(name="pscore", bufs=2,
                         space=bass.MemorySpace.PSUM) as pscore,
            tc.tile_pool(name="pacc", bufs=1,
                         space=bass.MemorySpace.PSUM) as pacc,
        ):
            ones_t = cpool.tile([128, 1], BF16, tag="ones")
            nc.gpsimd.memset(ones_t[:], 1.0)

            for b in range(b2):
                # small inputs ride the ACT ring so the sync ring leads with
                # the first big K chunk
                qt_sb = spool.tile([128, H], BF16, tag="qt_sb")
                nc.scalar.dma_start(out=qt_sb[:], in_=qt[b])
                bias_sb = spool.tile([128, t_tiles], F32, tag="bias_sb")
                nc.scalar.dma_start(out=bias_sb[:], in_=bias[b])

                ps_o = pacc.tile([H, H * D], F32, tag="ps_o")      # [16, 2048]
                ps_sums = pacc.tile([H, 1], F32, tag="ps_sums")

                kc_tiles = {}
                vc_tiles = {}

                def issue_chunk(ci, b=b):
                    sz, pi, _ = chunk_meta[ci]
                    kc = kpool.tile([128, sz * TILE_K], BF16, tag=f"kc{sz}")
                    nc.sync.dma_start(out=kc[:], in_=kparam[sz][b, pi])
                    vc = vpool.tile([128, sz * TILE_K], BF16, tag=f"vc{sz}")
                    nc.scalar.dma_start(out=vc[:], in_=vparam[sz][b, pi])
                    kc_tiles[ci] = kc
                    vc_tiles[ci] = vc

                def qk(t):
                    ci = tile2chunk[t]
                    _, _, ct0 = chunk_meta[ci]
                    kc = kc_tiles[ci]
                    ps_sc = pscore.tile([128, H], F32, tag="ps_sc")
                    for h in range(H):
                        o0 = (t - ct0) * TILE_K + h * 128
                        nc.tensor.matmul(
                            ps_sc[:, h:h + 1],
                            kc[:, o0:o0 + 128],
                            qt_sb[:, h:h + 1],
                            start=True, stop=True,
                        )
                    return ps_sc

                issue_chunk(0)
                if len(chunk_meta) > 1:
                    issue_chunk(1)
                ps_sc_t = qk(0)

                for t in range(t_tiles):
                    # stay one tile ahead on QK (and one chunk ahead on DMA)
                    if t + 1 < t_tiles:
                        if tile2chunk[t + 1] != tile2chunk[t]:
                            nci = tile2chunk[t + 1] + 1
                            if nci < len(chunk_meta) and nci not in kc_tiles:
                                issue_chunk(nci)
                        ps_sc_next = qk(t + 1)
                    else:
                        ps_sc_next = None

                    e_t = spool.tile([128, H], BF16, tag="e_t")
                    nc.scalar.activation(
                        e_t[:], ps_sc_t[:],
                        mybir.ActivationFunctionType.Exp,
                        bias=bias_sb[:, t:t + 1], scale=1.0,
                    )

                    first = t == 0
                    last = t == t_tiles - 1
                    ci = tile2chunk[t]
                    _, _, ct0 = chunk_meta[ci]
                    vc = vc_tiles[ci]
                    nc.tensor.matmul(ps_sums[:], e_t[:], ones_t[:],
                                     start=first, stop=last,
                                     skip_group_check=True)
                    for n in range(4):
                        o0 = (t - ct0) * TILE_K + n * 512
                        nc.tensor.matmul(
                            ps_o[:, n * 512:(n + 1) * 512],
                            e_t[:],
                            vc[:, o0:o0 + 512],
                            start=first, stop=last,
                            skip_group_check=True,
                        )
                    ps_sc_t = ps_sc_next

                recip = spool.tile([H, 1], F32, tag="recip")
                nc.vector.reciprocal(recip[:], ps_sums[:])
                o_full = spool.tile([H, H * D], F32, tag="o_full")
                # normalize on the (otherwise idle) ScalarE
                nc.scalar.mul(o_full[:], ps_o[:], recip[:])
                nc.sync.dma_start(out=out[b], in_=o_full[:])

    nc.compile()
    return nc


def prep_in_maps(q, k, v, k_cache, v_cache, block_tables, slot_mapping,
                 context_lens):
    """Host-side scatter + paged gather + per-core shard layouts."""
    q = np.asarray(q, np.float32)
    k = np.asarray(k, np.float32)
    v = np.asarray(v, np.float32)
    k_cache = np.asarray(k_cache, np.float32)
    v_cache = np.asarray(v_cache, np.float32)
    block_tables = np.asarray(block_tables, np.int32)
    slot_mapping = np.asarray(slot_mapping, np.int64)
    context_lens = np.asarray(context_lens, np.int32)

    nb, block_size, h, d = k_cache.shape
    # scatter the new token into the flat caches
    kc = k_cache.reshape(nb * block_size, h, d).copy()
    kc[slot_mapping] = k
    vc = v_cache.reshape(nb * block_size, h, d).copy()
    vc[slot_mapping] = v
    # paged gather -> [B, S, H, D]
    k_seq = kc.reshape(nb, block_size, h, d)[block_tables].reshape(B, S, h, d)
    v_seq = vc.reshape(nb, block_size, h, d)[block_tables].reshape(B, S, h, d)

    sizes = sorted(set(CHUNKS))
    kt_parts = {sz: [] for sz in sizes}
    v_parts = {sz: [] for sz in sizes}
    t0 = 0
    for sz in CHUNKS:
        s0, s1 = t0 * 128, (t0 + sz) * 128
        # K chunk: [B, sz*128, H, D] -> [B, 1, D, sz*H*128]
        kt_parts[sz].append(np.ascontiguousarray(
            k_seq[:, s0:s1].reshape(B, sz, 128, H, D)
            .transpose(0, 4, 1, 3, 2)).astype(NP_BF16)
            .reshape(B, 1, D, sz * TILE_K))
        # V chunk: [B, sz*128, H*D] -> [B, 1, 128, sz*H*D]
        v_parts[sz].append(np.ascontiguousarray(
            v_seq[:, s0:s1].reshape(B, sz, 128, H * D)
            .transpose(0, 2, 1, 3)).astype(NP_BF16)
            .reshape(B, 1, 128, sz * TILE_K))
        t0 += sz
    kt_host = {sz: np.concatenate(kt_parts[sz], axis=1) for sz in sizes}
    v_host = {sz: np.concatenate(v_parts[sz], axis=1) for sz in sizes}

    qt_host = np.ascontiguousarray(
        (q * SCALE).transpose(0, 2, 1)).astype(NP_BF16)
    s_idx = np.arange(S, dtype=np.int64)
    m = np.where(s_idx[None, :] < context_lens[:, None].astype(np.int64),
                 0.0, MASK_NEG).astype(np.float32)
    bias_host = np.ascontiguousarray(m.reshape(B, T, 128).transpose(0, 2, 1))

    in_maps = []
    for i in range(N_CORES):
        lo, hi = i * B2, (i + 1) * B2
        im = {"qt": np.ascontiguousarray(qt_host[lo:hi]),
              "bias": np.ascontiguousarray(bias_host[lo:hi])}
        for sz in sizes:
            im[f"kt{sz}"] = np.ascontiguousarray(kt_host[sz][lo:hi])
            im[f"vv{sz}"] = np.ascontiguousarray(v_host[sz][lo:hi])
        in_maps.append(im)
    return in_maps


_NC = None


def _get_nc():
    global _NC
    if _NC is None:
        _NC = build_nc()
    return _NC


def run(inputs, trace=False, **spmd_kwargs):
    """Run on hardware; returns (full_output, BassKernelResults)."""
    nc = _get_nc()
    in_maps = prep_in_maps(**inputs)
    res = run_bass_kernel_spmd(nc, in_maps, core_ids=list(range(N_CORES)),
                               trace=trace, **spmd_kwargs)
    out_full = np.concatenate([res.results[i]["out"] for i in range(N_CORES)],
                              axis=0).astype(np.float32)
    # extract the h'==h diagonal: [B, H, H*D] -> [B, H, D]
    hh = np.arange(H)
    out = out_full.reshape(B, H, H, D)[:, hh, hh, :]
    return np.ascontiguousarray(out), res


def kernel(**inputs) -> np.ndarray:
    out, _ = run(inputs, trace=False)
    return out
